# revision 2
# baseline (speedup 1.0000x reference)
"""Trainium2 Bass kernel for BilinearDecoder.

score = sigmoid( einsum('ed,ed->e', z[edges[0]] @ W, z[edges[1]]) )

Strategy (row-sorted edges, PE-windowed row side, transposed col gather):
  Host globally row-sorts the 131072 edges; core c owns the c-th
  contiguous slice of 16384.  Because edges are row-sorted, each group
  of 512 consecutive edges spans only ~45 distinct rows, so the host
  ships, per 512-edge group, a SW-row window of z (transposed, lhsT
  layout) — the row side needs NO per-edge gather at all.

  Phase 1: zwT2[d, n] = (z_window_rows @ W)^T computed on PE straight
  into SBUF (W-chunk lhsT x zt2 rhs, ACT casts f32->f16).

  Phase 2: cols arrive via dma_gather(transpose=True) from the DRAM z
  table: Ct[d, e] tiles [128, 4, 384] are ready-made matmul lhsT
  (384 idxs per gather: the transposed gather emits num_idxs/4+2 SBUF
  write descriptors per DMA engine and the SWDGE ring holds 128).
  Per 128-edge block: M[e, w] = sum_d Ct[d, e] * zwT2[d, w0_g + w]
  (4 accumulating matmuls, free dim SW).  score_e = M[e, rl_e] is
  selected by a host-built one-hot mask: DVE tensor_mul + segmented
  tensor_reduce over [128, 3, SW] per gather chunk.  One sigmoid, one
  DMA out; host unsorts globally.

  vs the previous kernel this removes the 16.8MB/core SBUF row-gather
  entirely and moves the per-edge dot from DVE/ACT onto the PE, so the
  critical path is a single 16.8MB/core transposed col-gather stream.
"""

import sys

if "/opt/trn_rl_repo" not in sys.path:
    sys.path.insert(0, "/opt/trn_rl_repo")

import numpy as np

N_NODES = 10000
N_PAD = 10240
W_DIM = 512
N_EDGES = 131072
N_CORES = 8
EC = N_EDGES // N_CORES     # 16384 edges per core
NBLK = EC // 128            # 128 blocks of 128 edges
G_EDGES = 512               # edges per row-window group
NG = EC // G_EDGES          # 32 groups
TNI = 384                   # idxs per transpose gather (ring limit)
NGATH = (EC + TNI - 1) // TNI  # 43 gathers (last one 2/3 real)
EC_PAD = NGATH * TNI        # 16512
NQ = 4

_cache = {}


def _build(SW):
    import concourse.bacc as bacc
    import concourse.tile as tile
    from concourse import mybir
    from concourse.bass import InstructionNameOrderedSet

    f32 = mybir.dt.float32
    f16 = mybir.dt.float16
    i16 = mybir.dt.int16
    NW = NG * SW  # zwT2 columns

    nc = bacc.Bacc(
        "TRN2",
        target_bir_lowering=False,
        debug=False,
        num_devices=N_CORES,
        num_swdge_queues=NQ,
    )
    # zt2[p, k, g*SW+j] = z[w0_g + j, k*128+p]: per-group row-window
    # segments, transposed into lhsT-panel layout on host.
    zt2 = nc.dram_tensor("zt2", [128, 4, NW], f16, kind="ExternalInput")
    ztbl = nc.dram_tensor("ztbl", [N_PAD, W_DIM], f16, kind="ExternalInput")
    w = nc.dram_tensor("w", [W_DIM, W_DIM], f16, kind="ExternalInput")
    cidx = nc.dram_tensor("cidx", [128, EC_PAD // 16], i16, kind="ExternalInput")
    masks = nc.dram_tensor("masks", [128, NBLK, SW], f16, kind="ExternalInput")
    out = nc.dram_tensor("scores", [128, NBLK], f32, kind="ExternalOutput")

    with tile.TileContext(nc) as tc:
        with (
            tc.tile_pool(name="wpool", bufs=1) as wpool,
            tc.tile_pool(name="zin", bufs=1) as zinpool,
            tc.tile_pool(name="zwt", bufs=1) as zwpool,
            tc.tile_pool(name="ps1", bufs=2, space="PSUM") as ps1pool,
            tc.tile_pool(name="psm", bufs=5, space="PSUM") as psmpool,
            tc.tile_pool(name="idx", bufs=1) as idxpool,
            tc.tile_pool(name="cgath", bufs=8) as cpool,
            tc.tile_pool(name="scr", bufs=6) as spool,
            tc.tile_pool(name="misc", bufs=1) as mpool,
        ):
            # Issue order = latency: cidx gates the lead col gathers,
            # then zt2/W gate phase 1; masks are needed only by the
            # first DVE select (~15us in).
            cidx_sb = idxpool.tile([128, EC_PAD // 16], i16, tag="cidx")
            nc.sync.dma_start(cidx_sb[:], cidx[:])
            zt2a = zinpool.tile([128, 4, NW // 2], f16, tag="zt2a")
            nc.sync.dma_start(zt2a[:], zt2[:, :, : NW // 2])
            w_tiles = []
            for k in range(4):
                wt = wpool.tile([128, W_DIM], f16, tag=f"w{k}")
                nc.sync.dma_start(wt[:], w[k * 128:(k + 1) * 128, :])
                w_tiles.append(wt)
            zt2b = zinpool.tile([128, 4, NW // 2], f16, tag="zt2b")
            nc.sync.dma_start(zt2b[:], zt2[:, :, NW // 2:])
            mask_sb = mpool.tile([128, NBLK, SW], f16, tag="masks")
            nc.sync.dma_start(mask_sb[:], masks[:])

            scores = mpool.tile([128, NBLK], f32, tag="scores")

            # Dummy sigmoid first: the ACT-table pass picks a function
            # set containing BOTH sigmoid and copy, so the final sigmoid
            # needs no table reload on the critical tail.
            warm = mpool.tile([128, 1], f32, tag="warm")
            nc.scalar.activation(
                warm[:], w_tiles[0][:, 0:1], mybir.ActivationFunctionType.Sigmoid
            )

            # SBUF-resident zwT2[p, t, n] = (z2 @ W)[n, t*128+p]
            zwt_sb = zwpool.tile([128, 4, NW], f16, tag="zwt")

            # ---- col gathers: nosync-chained, rotating queues ----
            gstate = {"pos": 0, "prev": None}

            def emit_gather(ch):
                ct = cpool.tile([128, 4, TNI], f16, tag="ct")
                icol = slice(ch * (TNI // 16), (ch + 1) * (TNI // 16))
                inst = nc.gpsimd.dma_gather(
                    ct[:], ztbl[:], cidx_sb[:, icol], TNI, TNI, W_DIM,
                    transpose=True, queue_num=gstate["pos"] % NQ,
                )
                if gstate["prev"] is not None:
                    deps = InstructionNameOrderedSet()
                    deps.add(gstate["prev"])
                    inst.ins.add_nosync_dependencies_from(deps)
                gstate["prev"] = inst.ins.name
                gstate["pos"] += 1
                return ct

            LEAD = 6
            ct_tiles = {}
            for ch in range(LEAD):
                ct_tiles[ch] = emit_gather(ch)

            # ---- Phase 1: zwT2 = (z2 @ W)^T ----
            # out[d, n] = sum_k W[k, d] z2[n, k]: lhsT = W k-chunk
            # [128, d-tile], rhs = zt2[:, k, n-chunk].
            NCH1 = 4          # n-chunks of NW//4 columns
            CW = NW // NCH1   # 384 for SW=48 (PSUM bank: 1536B)
            for m in range(NCH1):
                src = zt2a if m < NCH1 // 2 else zt2b
                off = m * CW - (0 if m < NCH1 // 2 else NW // 2)
                for t in range(4):
                    ps = ps1pool.tile([128, CW], f32, tag="ps1")
                    for k in range(4):
                        nc.tensor.matmul(
                            ps[:],
                            lhsT=w_tiles[k][:, t * 128:(t + 1) * 128],
                            rhs=src[:, k, off:off + CW],
                            start=(k == 0),
                            stop=(k == 3),
                        )
                    nc.scalar.activation(
                        zwt_sb[:, t, m * CW:(m + 1) * CW],
                        ps[:],
                        mybir.ActivationFunctionType.Copy,
                    )

            # ---- Phase 2: per gather chunk: 3 blocks x 4 matmuls,
            # DVE mask-mul + segmented reduce ----
            def emit_dot(ch):
                ct = ct_tiles.pop(ch)
                b0 = (ch * TNI) // 128  # first block in this gather
                nblk = min(3, NBLK - b0)
                if nblk <= 0:
                    return
                M = psmpool.tile([128, 3, SW], f32, tag="M")
                for j in range(nblk):
                    b = b0 + j
                    g = b // 4
                    for k in range(4):
                        nc.tensor.matmul(
                            M[:, j, :],
                            lhsT=ct[:, k, j * 128:(j + 1) * 128],
                            rhs=zwt_sb[:, k, g * SW:(g + 1) * SW],
                            start=(k == 0),
                            stop=(k == 3),
                        )
                scr = spool.tile([128, 3, SW], f16, tag="scr")
                nc.vector.tensor_mul(
                    scr[:, :nblk, :], M[:, :nblk, :], mask_sb[:, b0:b0 + nblk, :]
                )
                nc.vector.tensor_reduce(
                    scores[:, b0:b0 + nblk],
                    scr[:, :nblk, :],
                    mybir.AxisListType.X,
                    mybir.AluOpType.add,
                )

            for ch in range(NGATH):
                if ch + LEAD < NGATH:
                    ct_tiles[ch + LEAD] = emit_gather(ch + LEAD)
                emit_dot(ch)

            sig = mpool.tile([128, NBLK], f32, tag="sig")
            nc.scalar.activation(
                sig[:], scores[:], mybir.ActivationFunctionType.Sigmoid
            )
            nc.sync.dma_start(out[:], sig[:])

    nc.compile()
    return nc


def _get_nc(SW):
    key = f"nc_{SW}"
    if key not in _cache:
        _cache[key] = _build(SW)
    return _cache[key]


def _wrap_idx(idx):
    """int16 indices -> [128, n/16] layout: index i at [i%16, i//16],
    replicated across the 8 GPSIMD core groups (16 partitions each)."""
    blk = idx.reshape(-1, 16).T.astype(np.int16)
    return np.ascontiguousarray(np.tile(blk, (8, 1)))


def kernel(z, batch_edges, W, _profile=False):
    from concourse.bass_utils import run_bass_kernel_spmd

    z = np.asarray(z, dtype=np.float32)
    W = np.asarray(W, dtype=np.float32)
    be = np.asarray(batch_edges)

    z_pad = np.zeros((N_PAD, W_DIM), dtype=np.float32)
    z_pad[:N_NODES] = z
    z16 = z_pad.astype(np.float16)
    w_np = W.astype(np.float16)

    rows_all = be[0].astype(np.int64)
    cols_all = be[1].astype(np.int64)
    glob_order = np.argsort(rows_all, kind="stable")

    # Shared window size across cores/groups (baked into the program).
    SW = 16
    per_core = []
    for c in range(N_CORES):
        sel = glob_order[c * EC:(c + 1) * EC]
        r_s = rows_all[sel]
        c_s = cols_all[sel]
        rg = r_s.reshape(NG, G_EDGES)
        w0 = rg[:, 0]
        span = int((rg[:, -1] - w0 + 1).max())
        SW = max(SW, span)
        per_core.append((r_s, c_s, w0))
    SW = (SW + 15) // 16 * 16
    assert SW <= 64, f"row-window {SW} exceeds 64 (PSUM bank limit)"

    in_maps = []
    for c in range(N_CORES):
        r_s, c_s, w0 = per_core[c]
        # zt2 segments: group g = z rows [w0_g, w0_g+SW), transposed
        zt2 = np.zeros((128, 4, NG * SW), np.float16)
        for g in range(NG):
            a = int(w0[g])
            seg = z16[a:a + SW]
            if seg.shape[0] < SW:  # clip at table end (can't happen: rows<10000)
                seg = np.pad(seg, ((0, SW - seg.shape[0]), (0, 0)))
            zt2[:, :, g * SW:(g + 1) * SW] = seg.reshape(SW, 4, 128).transpose(2, 1, 0)
        # one-hot masks: mask[p, b, w] = (rl[b*128+p] == w)
        rl = (r_s - np.repeat(w0, G_EDGES)).astype(np.int64)  # [EC] in [0, SW)
        assert rl.min() >= 0 and rl.max() < SW
        mk = np.zeros((NBLK * 128, SW), np.float16)
        mk[np.arange(EC), rl] = 1.0
        mk = np.ascontiguousarray(
            mk.reshape(NBLK, 128, SW).transpose(1, 0, 2))  # [128, NBLK, SW]
        cpad = np.zeros(EC_PAD, np.int16)
        cpad[:EC] = c_s.astype(np.int16)
        in_maps.append({
            "zt2": np.ascontiguousarray(zt2),
            "ztbl": z16,
            "w": w_np,
            "cidx": _wrap_idx(cpad),
            "masks": mk,
        })

    nc = _get_nc(SW)
    kwargs = {"trace": True} if _profile else {}
    res = run_bass_kernel_spmd(nc, in_maps, core_ids=list(range(N_CORES)), **kwargs)
    _cache["last_res"] = res

    chunks = []
    for c in range(N_CORES):
        sc = res.results[c]["scores"]  # [128, NBLK], edge b*128+p at [p, b]
        chunks.append(np.ascontiguousarray(sc.T).reshape(-1))
    full = np.empty(N_EDGES, dtype=np.float32)
    full[glob_order] = np.concatenate(chunks)
    return full


# revision 6
# speedup vs baseline: 1.1101x; 1.1101x over previous
"""Trainium2 Bass kernel for BilinearDecoder.

score = sigmoid( einsum('ed,ed->e', z[edges[0]] @ W, z[edges[1]]) )

Strategy (row-sorted edges, PE-windowed row side, transposed col gather):
  Host globally row-sorts the 131072 edges; core c owns the c-th
  contiguous slice of 16384.  Because edges are row-sorted, each group
  of 512 consecutive edges spans only ~45 distinct rows, so the host
  ships, per 512-edge group, a SW-row window of z (transposed, lhsT
  layout) — the row side needs NO per-edge gather at all.

  Phase 1: zwT2[d, n] = (z_window_rows @ W)^T computed on PE straight
  into SBUF (W-chunk lhsT x zt2 rhs, ACT casts f32->f16).

  Phase 2: cols arrive via dma_gather(transpose=True) from the DRAM z
  table: Ct[d, e] tiles [128, 4, 384] are ready-made matmul lhsT
  (384 idxs per gather: the transposed gather emits num_idxs/4+2 SBUF
  write descriptors per DMA engine and the SWDGE ring holds 128).
  Per 128-edge block: M[e, w] = sum_d Ct[d, e] * zwT2[d, w0_g + w]
  (4 accumulating matmuls, free dim SW).  score_e = M[e, rl_e] is
  selected by a host-built one-hot mask: DVE tensor_mul + segmented
  tensor_reduce over [128, 3, SW] per gather chunk.  One sigmoid, one
  DMA out; host unsorts globally.

  vs the previous kernel this removes the 16.8MB/core SBUF row-gather
  entirely and moves the per-edge dot from DVE/ACT onto the PE, so the
  critical path is a single 16.8MB/core transposed col-gather stream.
"""

import sys

if "/opt/trn_rl_repo" not in sys.path:
    sys.path.insert(0, "/opt/trn_rl_repo")

import numpy as np

N_NODES = 10000
N_PAD = 10240
W_DIM = 512
N_EDGES = 131072
N_CORES = 8
EC = N_EDGES // N_CORES     # 16384 edges per core
NBLK = EC // 128            # 128 blocks of 128 edges
G_EDGES = 512               # edges per row-window group
NG = EC // G_EDGES          # 32 groups
TNI = 384                   # idxs per transpose gather (ring limit)
NGATH = (EC + TNI - 1) // TNI  # 43 gathers (last one 2/3 real)
EC_PAD = NGATH * TNI        # 16512
NQ = 4

_cache = {}


def _build(SW):
    import concourse.bacc as bacc
    import concourse.tile as tile
    from concourse import library_config, mybir
    from concourse.bass import InstructionNameOrderedSet

    f32 = mybir.dt.float32
    f16 = mybir.dt.float16
    i16 = mybir.dt.int16
    NW = NG * SW  # zwT2 columns

    nc = bacc.Bacc(
        "TRN2",
        target_bir_lowering=False,
        debug=False,
        num_devices=N_CORES,
        num_swdge_queues=NQ,
    )
    # zt2[p, k, g*SW+j] = z[w0_g + j, k*128+p]: per-group row-window
    # segments, transposed into lhsT-panel layout on host.
    zt2 = nc.dram_tensor("zt2", [128, 4, NW], f16, kind="ExternalInput")
    ztbl = nc.dram_tensor("ztbl", [N_PAD, W_DIM], f16, kind="ExternalInput")
    w = nc.dram_tensor("w", [W_DIM, W_DIM], f16, kind="ExternalInput")
    cidx = nc.dram_tensor("cidx", [128, EC_PAD // 16], i16, kind="ExternalInput")
    masks = nc.dram_tensor("masks", [128, NBLK, SW], f16, kind="ExternalInput")
    out = nc.dram_tensor("scores", [128, NBLK], f32, kind="ExternalOutput")

    with tile.TileContext(nc) as tc:
        with (
            tc.tile_pool(name="wpool", bufs=1) as wpool,
            tc.tile_pool(name="zin", bufs=1) as zinpool,
            tc.tile_pool(name="zwt", bufs=1) as zwpool,
            tc.tile_pool(name="ps1", bufs=2, space="PSUM") as ps1pool,
            tc.tile_pool(name="psm", bufs=5, space="PSUM") as psmpool,
            tc.tile_pool(name="idx", bufs=1) as idxpool,
            tc.tile_pool(name="cgath", bufs=14) as cpool,
            tc.tile_pool(name="scr", bufs=8) as spool,
            tc.tile_pool(name="misc", bufs=1) as mpool,
        ):
            # The SWDGE gather ucode lives in the 'mlp' GPSIMD library;
            # loading it takes ~15us (image DMA).  Kick it off first so
            # it overlaps the input DMAs instead of gating gather 0.
            nc.gpsimd.load_library(library_config.mlp)

            # Issue order = latency: cidx gates the lead col gathers,
            # then zt2/W gate phase 1; masks are needed only by the
            # first DVE select (~15us in).
            cidx_sb = idxpool.tile([128, EC_PAD // 16], i16, tag="cidx")
            nc.sync.dma_start(cidx_sb[:], cidx[:])
            zt2a = zinpool.tile([128, 4, NW // 2], f16, tag="zt2a")
            nc.sync.dma_start(zt2a[:], zt2[:, :, : NW // 2])
            w_tiles = []
            for k in range(4):
                wt = wpool.tile([128, W_DIM], f16, tag=f"w{k}")
                nc.sync.dma_start(wt[:], w[k * 128:(k + 1) * 128, :])
                w_tiles.append(wt)
            zt2b = zinpool.tile([128, 4, NW // 2], f16, tag="zt2b")
            nc.sync.dma_start(zt2b[:], zt2[:, :, NW // 2:])
            mask_sb = mpool.tile([128, NBLK, SW], f16, tag="masks")
            nc.sync.dma_start(mask_sb[:], masks[:])

            scores = mpool.tile([128, NBLK], f32, tag="scores")

            # Dummy sigmoid first: the ACT-table pass picks a function
            # set containing BOTH sigmoid and copy, so the final sigmoid
            # needs no table reload on the critical tail.
            warm = mpool.tile([128, 1], f32, tag="warm")
            nc.scalar.activation(
                warm[:], w_tiles[0][:, 0:1], mybir.ActivationFunctionType.Sigmoid
            )

            # SBUF-resident zwT2[p, t, n] = (z2 @ W)[n, t*128+p]
            zwt_sb = zwpool.tile([128, 4, NW], f16, tag="zwt")

            # ---- col gathers: nosync-chained, rotating queues ----
            gstate = {"pos": 0, "prev": None}

            def emit_gather(ch):
                ct = cpool.tile([128, 4, TNI], f16, tag="ct")
                icol = slice(ch * (TNI // 16), (ch + 1) * (TNI // 16))
                inst = nc.gpsimd.dma_gather(
                    ct[:], ztbl[:], cidx_sb[:, icol], TNI, TNI, W_DIM,
                    transpose=True, queue_num=gstate["pos"] % NQ,
                )
                if gstate["prev"] is not None:
                    deps = InstructionNameOrderedSet()
                    deps.add(gstate["prev"])
                    inst.ins.add_nosync_dependencies_from(deps)
                gstate["prev"] = inst.ins.name
                gstate["pos"] += 1
                return ct

            LEAD = 10
            ct_tiles = {}
            for ch in range(LEAD):
                ct_tiles[ch] = emit_gather(ch)

            # ---- Phase 1: zwT2 = (z2 @ W)^T ----
            # out[d, n] = sum_k W[k, d] z2[n, k]: lhsT = W k-chunk
            # [128, d-tile], rhs = zt2[:, k, n-chunk].
            NCH1 = 4          # n-chunks of NW//4 columns
            CW = NW // NCH1   # 384 for SW=48 (PSUM bank: 1536B)
            for m in range(NCH1):
                src = zt2a if m < NCH1 // 2 else zt2b
                off = m * CW - (0 if m < NCH1 // 2 else NW // 2)
                for t in range(4):
                    ps = ps1pool.tile([128, CW], f32, tag="ps1")
                    for k in range(4):
                        nc.tensor.matmul(
                            ps[:],
                            lhsT=w_tiles[k][:, t * 128:(t + 1) * 128],
                            rhs=src[:, k, off:off + CW],
                            start=(k == 0),
                            stop=(k == 3),
                        )
                    nc.scalar.activation(
                        zwt_sb[:, t, m * CW:(m + 1) * CW],
                        ps[:],
                        mybir.ActivationFunctionType.Copy,
                    )

            # ---- Phase 2: per gather chunk: 3 blocks x 4 matmuls,
            # DVE mask-mul + segmented reduce ----
            def emit_dot(ch):
                ct = ct_tiles.pop(ch)
                b0 = (ch * TNI) // 128  # first block in this gather
                nblk = min(3, NBLK - b0)
                if nblk <= 0:
                    return
                M = psmpool.tile([128, 3, SW], f32, tag="M")
                for j in range(nblk):
                    b = b0 + j
                    g = b // 4
                    for k in range(4):
                        nc.tensor.matmul(
                            M[:, j, :],
                            lhsT=ct[:, k, j * 128:(j + 1) * 128],
                            rhs=zwt_sb[:, k, g * SW:(g + 1) * SW],
                            start=(k == 0),
                            stop=(k == 3),
                        )
                scr = spool.tile([128, 3, SW], f16, tag="scr")
                nc.vector.tensor_mul(
                    scr[:, :nblk, :], M[:, :nblk, :], mask_sb[:, b0:b0 + nblk, :]
                )
                nc.vector.tensor_reduce(
                    scores[:, b0:b0 + nblk],
                    scr[:, :nblk, :],
                    mybir.AxisListType.X,
                    mybir.AluOpType.add,
                )

            for ch in range(NGATH):
                if ch + LEAD < NGATH:
                    ct_tiles[ch + LEAD] = emit_gather(ch + LEAD)
                emit_dot(ch)

            sig = mpool.tile([128, NBLK], f32, tag="sig")
            nc.scalar.activation(
                sig[:], scores[:], mybir.ActivationFunctionType.Sigmoid
            )
            nc.sync.dma_start(out[:], sig[:])

    nc.compile()
    return nc


def _get_nc(SW):
    key = f"nc_{SW}"
    if key not in _cache:
        _cache[key] = _build(SW)
    return _cache[key]


def _wrap_idx(idx):
    """int16 indices -> [128, n/16] layout: index i at [i%16, i//16],
    replicated across the 8 GPSIMD core groups (16 partitions each)."""
    blk = idx.reshape(-1, 16).T.astype(np.int16)
    return np.ascontiguousarray(np.tile(blk, (8, 1)))


def kernel(z, batch_edges, W, _profile=False):
    from concourse.bass_utils import run_bass_kernel_spmd

    z = np.asarray(z, dtype=np.float32)
    W = np.asarray(W, dtype=np.float32)
    be = np.asarray(batch_edges)

    z_pad = np.zeros((N_PAD, W_DIM), dtype=np.float32)
    z_pad[:N_NODES] = z
    z16 = z_pad.astype(np.float16)
    w_np = W.astype(np.float16)

    rows_all = be[0].astype(np.int64)
    cols_all = be[1].astype(np.int64)
    glob_order = np.argsort(rows_all, kind="stable")

    # Shared window size across cores/groups (baked into the program).
    SW = 16
    per_core = []
    for c in range(N_CORES):
        sel = glob_order[c * EC:(c + 1) * EC]
        r_s = rows_all[sel]
        c_s = cols_all[sel]
        rg = r_s.reshape(NG, G_EDGES)
        w0 = rg[:, 0]
        span = int((rg[:, -1] - w0 + 1).max())
        SW = max(SW, span)
        per_core.append((r_s, c_s, w0))
    SW = (SW + 15) // 16 * 16
    assert SW <= 64, f"row-window {SW} exceeds 64 (PSUM bank limit)"

    in_maps = []
    for c in range(N_CORES):
        r_s, c_s, w0 = per_core[c]
        # zt2 segments: group g = z rows [w0_g, w0_g+SW), transposed
        zt2 = np.zeros((128, 4, NG * SW), np.float16)
        for g in range(NG):
            a = int(w0[g])
            seg = z16[a:a + SW]
            if seg.shape[0] < SW:  # clip at table end (can't happen: rows<10000)
                seg = np.pad(seg, ((0, SW - seg.shape[0]), (0, 0)))
            zt2[:, :, g * SW:(g + 1) * SW] = seg.reshape(SW, 4, 128).transpose(2, 1, 0)
        # one-hot masks: mask[p, b, w] = (rl[b*128+p] == w)
        rl = (r_s - np.repeat(w0, G_EDGES)).astype(np.int64)  # [EC] in [0, SW)
        assert rl.min() >= 0 and rl.max() < SW
        mk = np.zeros((NBLK * 128, SW), np.float16)
        mk[np.arange(EC), rl] = 1.0
        mk = np.ascontiguousarray(
            mk.reshape(NBLK, 128, SW).transpose(1, 0, 2))  # [128, NBLK, SW]
        cpad = np.zeros(EC_PAD, np.int16)
        cpad[:EC] = c_s.astype(np.int16)
        in_maps.append({
            "zt2": np.ascontiguousarray(zt2),
            "ztbl": z16,
            "w": w_np,
            "cidx": _wrap_idx(cpad),
            "masks": mk,
        })

    nc = _get_nc(SW)
    kwargs = {"trace": True} if _profile else {}
    res = run_bass_kernel_spmd(nc, in_maps, core_ids=list(range(N_CORES)), **kwargs)
    _cache["last_res"] = res

    chunks = []
    for c in range(N_CORES):
        sc = res.results[c]["scores"]  # [128, NBLK], edge b*128+p at [p, b]
        chunks.append(np.ascontiguousarray(sc.T).reshape(-1))
    full = np.empty(N_EDGES, dtype=np.float32)
    full[glob_order] = np.concatenate(chunks)
    return full


# revision 13
# speedup vs baseline: 1.1533x; 1.0390x over previous
"""Trainium2 Bass kernel for BilinearDecoder.

score = sigmoid( einsum('ed,ed->e', z[edges[0]] @ W, z[edges[1]]) )

Strategy (row-sorted edges, PE-windowed row side, transposed col gather):
  Host globally row-sorts the 131072 edges; core c owns the c-th
  contiguous slice of 16384.  Because edges are row-sorted, each group
  of 512 consecutive edges spans only ~45 distinct rows, so the host
  ships, per 512-edge group, a SW-row window of z (transposed, lhsT
  layout) — the row side needs NO per-edge gather at all.

  Phase 1: zwT2[d, n] = (z_window_rows @ W)^T computed on PE straight
  into SBUF (W-chunk lhsT x zt2 rhs, ACT casts f32->f16).

  Phase 2: cols arrive via dma_gather(transpose=True) from the DRAM z
  table: Ct[d, e] tiles [128, 4, 384] are ready-made matmul lhsT
  (384 idxs per gather: the transposed gather emits num_idxs/4+2 SBUF
  write descriptors per DMA engine and the SWDGE ring holds 128).
  Per 128-edge block: M[e, w] = sum_d Ct[d, e] * zwT2[d, w0_g + w]
  (4 accumulating matmuls, free dim SW).  score_e = M[e, rl_e] is
  selected by a host-built one-hot mask: DVE tensor_mul + segmented
  tensor_reduce over [128, 3, SW] per gather chunk.  One sigmoid, one
  DMA out; host unsorts globally.

  vs the previous kernel this removes the 16.8MB/core SBUF row-gather
  entirely and moves the per-edge dot from DVE/ACT onto the PE, so the
  critical path is a single 16.8MB/core transposed col-gather stream.
"""

import sys

if "/opt/trn_rl_repo" not in sys.path:
    sys.path.insert(0, "/opt/trn_rl_repo")

import numpy as np

N_NODES = 10000
N_PAD = 10240
W_DIM = 512
N_EDGES = 131072
N_CORES = 8
EC = N_EDGES // N_CORES     # 16384 edges per core
NBLK = EC // 128            # 128 blocks of 128 edges
G_EDGES = 512               # edges per row-window group
NG = EC // G_EDGES          # 32 groups
TNI = 384                   # idxs per transpose gather (ring limit)
NGATH = (EC + TNI - 1) // TNI  # 43 gathers (last one 2/3 real)
EC_PAD = NGATH * TNI        # 16512
NQ = 4
KPRE = 12                   # chunks pre-gathered on host, DMA'd dense
                            # (covers the ~21us GPSIMD lib-load window)

_cache = {}


def _build(SW):
    import concourse.bacc as bacc
    import concourse.tile as tile
    from concourse import library_config, mybir
    from concourse.bass import InstructionNameOrderedSet

    f32 = mybir.dt.float32
    f16 = mybir.dt.float16
    i16 = mybir.dt.int16
    NW = NG * SW  # zwT2 columns

    nc = bacc.Bacc(
        "TRN2",
        target_bir_lowering=False,
        debug=False,
        num_devices=N_CORES,
        num_swdge_queues=NQ,
    )
    # zt2[p, k, g*SW+j] = z[w0_g + j, k*128+p]: per-group row-window
    # segments, transposed into lhsT-panel layout on host.
    zt2 = nc.dram_tensor("zt2", [128, 4, NW], f16, kind="ExternalInput")
    ztbl = nc.dram_tensor("ztbl", [N_PAD, W_DIM], f16, kind="ExternalInput")
    w = nc.dram_tensor("w", [W_DIM, W_DIM], f16, kind="ExternalInput")
    cidx = nc.dram_tensor("cidx", [128, EC_PAD // 16], i16, kind="ExternalInput")
    masks = nc.dram_tensor("masks", [128, NBLK, SW], f16, kind="ExternalInput")
    ctpre = nc.dram_tensor("ctpre", [KPRE, 128, 4, TNI], f16, kind="ExternalInput")
    out = nc.dram_tensor("scores", [128, NBLK], f32, kind="ExternalOutput")

    with tile.TileContext(nc) as tc:
        with (
            tc.tile_pool(name="wpool", bufs=1) as wpool,
            tc.tile_pool(name="zin", bufs=1) as zinpool,
            tc.tile_pool(name="zwt", bufs=1) as zwpool,
            tc.tile_pool(name="ps1", bufs=2, space="PSUM") as ps1pool,
            tc.tile_pool(name="psm", bufs=5, space="PSUM") as psmpool,
            tc.tile_pool(name="idx", bufs=1) as idxpool,
            tc.tile_pool(name="cgath", bufs=20) as cpool,
            tc.tile_pool(name="scr", bufs=8) as spool,
            tc.tile_pool(name="misc", bufs=1) as mpool,
        ):
            # The SWDGE gather ucode lives in the 'mlp' GPSIMD library;
            # loading it takes ~15us (image DMA).  Kick it off first so
            # it overlaps the input DMAs instead of gating gather 0.
            nc.gpsimd.load_library(library_config.mlp)

            # Issue order = latency: cidx gates the lead col gathers,
            # then zt2/W gate phase 1; masks are needed only by the
            # first DVE select (~15us in).
            zt2a = zinpool.tile([128, 4, NW // 2], f16, tag="zt2a")
            nc.sync.dma_start(zt2a[:], zt2[:, :, : NW // 2])
            w_tiles = []
            for k in range(4):
                wt = wpool.tile([128, W_DIM], f16, tag=f"w{k}")
                nc.sync.dma_start(wt[:], w[k * 128:(k + 1) * 128, :])
                w_tiles.append(wt)
            cidx_sb = idxpool.tile([128, EC_PAD // 16], i16, tag="cidx")
            nc.sync.dma_start(cidx_sb[:], cidx[:])
            zt2b = zinpool.tile([128, 4, NW // 2], f16, tag="zt2b")
            nc.sync.dma_start(zt2b[:], zt2[:, :, NW // 2:])
            mask_sb = mpool.tile([128, NBLK, SW], f16, tag="masks")
            nc.sync.dma_start(mask_sb[:], masks[:])

            scores = mpool.tile([128, NBLK], f32, tag="scores")

            # Dummy sigmoid first: the ACT-table pass picks a function
            # set containing BOTH sigmoid and copy, so the final sigmoid
            # needs no table reload on the critical tail.
            warm = mpool.tile([128, 1], f32, tag="warm")
            nc.scalar.activation(
                warm[:], w_tiles[0][:, 0:1], mybir.ActivationFunctionType.Sigmoid
            )

            # SBUF-resident zwT2[p, t, n] = (z2 @ W)[n, t*128+p]
            zwt_sb = zwpool.tile([128, 4, NW], f16, tag="zwt")

            # ---- col gathers: nosync-chained, rotating queues ----
            gstate = {"pos": 0, "prev": None}

            def emit_gather(ch):
                ct = cpool.tile([128, 4, TNI], f16, tag="ct")
                icol = slice(ch * (TNI // 16), (ch + 1) * (TNI // 16))
                inst = nc.gpsimd.dma_gather(
                    ct[:], ztbl[:], cidx_sb[:, icol], TNI, TNI, W_DIM,
                    transpose=True, queue_num=gstate["pos"] % NQ,
                )
                if gstate["prev"] is not None:
                    deps = InstructionNameOrderedSet()
                    deps.add(gstate["prev"])
                    inst.ins.add_nosync_dependencies_from(deps)
                gstate["prev"] = inst.ins.name
                gstate["pos"] += 1
                return ct

            # First KPRE col tiles come in dense over HWDGE (host
            # pre-gathered + pre-transposed), hiding the lib load.
            ct_tiles = {}
            for ch in range(KPRE):
                ct = cpool.tile([128, 4, TNI], f16, tag="ct")
                nc.sync.dma_start(ct[:], ctpre[ch])
                ct_tiles[ch] = ct

            LEAD = 10
            for ch in range(KPRE, KPRE + LEAD):
                ct_tiles[ch] = emit_gather(ch)

            # ---- Phase 1: zwT2 = (z2 @ W)^T ----
            # out[d, n] = sum_k W[k, d] z2[n, k]: lhsT = W k-chunk
            # [128, d-tile], rhs = zt2[:, k, n-chunk].
            NCH1 = 4          # n-chunks of NW//4 columns
            CW = NW // NCH1   # 384 for SW=48 (PSUM bank: 1536B)
            for m in range(NCH1):
                src = zt2a if m < NCH1 // 2 else zt2b
                off = m * CW - (0 if m < NCH1 // 2 else NW // 2)
                for t in range(4):
                    ps = ps1pool.tile([128, CW], f32, tag="ps1")
                    for k in range(4):
                        nc.tensor.matmul(
                            ps[:],
                            lhsT=w_tiles[k][:, t * 128:(t + 1) * 128],
                            rhs=src[:, k, off:off + CW],
                            start=(k == 0),
                            stop=(k == 3),
                        )
                    nc.scalar.activation(
                        zwt_sb[:, t, m * CW:(m + 1) * CW],
                        ps[:],
                        mybir.ActivationFunctionType.Copy,
                    )

            # ---- Phase 2: per gather chunk: 3 blocks x 4 matmuls,
            # DVE mask-mul + segmented reduce ----
            def emit_dot(ch):
                ct = ct_tiles.pop(ch)
                b0 = (ch * TNI) // 128  # first block in this gather
                nblk = min(3, NBLK - b0)
                if nblk <= 0:
                    return
                M = psmpool.tile([128, 3, SW], f32, tag="M")
                for j in range(nblk):
                    b = b0 + j
                    g = b // 4
                    for k in range(4):
                        nc.tensor.matmul(
                            M[:, j, :],
                            lhsT=ct[:, k, j * 128:(j + 1) * 128],
                            rhs=zwt_sb[:, k, g * SW:(g + 1) * SW],
                            start=(k == 0),
                            stop=(k == 3),
                        )
                scr = spool.tile([128, 3, SW], f16, tag="scr")
                nc.vector.tensor_mul(
                    scr[:, :nblk, :], M[:, :nblk, :], mask_sb[:, b0:b0 + nblk, :]
                )
                nc.vector.tensor_reduce(
                    scores[:, b0:b0 + nblk],
                    scr[:, :nblk, :],
                    mybir.AxisListType.X,
                    mybir.AluOpType.add,
                )

            for ch in range(NGATH):
                if ch + KPRE + LEAD < NGATH:
                    ct_tiles[ch + KPRE + LEAD] = emit_gather(ch + KPRE + LEAD)
                emit_dot(ch)

            sig = mpool.tile([128, NBLK], f32, tag="sig")
            nc.scalar.activation(
                sig[:], scores[:], mybir.ActivationFunctionType.Sigmoid
            )
            nc.sync.dma_start(out[:], sig[:])

    nc.compile()
    return nc


def _get_nc(SW):
    key = f"nc_{SW}"
    if key not in _cache:
        _cache[key] = _build(SW)
    return _cache[key]


def _wrap_idx(idx):
    """int16 indices -> [128, n/16] layout: index i at [i%16, i//16],
    replicated across the 8 GPSIMD core groups (16 partitions each)."""
    blk = idx.reshape(-1, 16).T.astype(np.int16)
    return np.ascontiguousarray(np.tile(blk, (8, 1)))


def kernel(z, batch_edges, W, _profile=False):
    from concourse.bass_utils import run_bass_kernel_spmd

    z = np.asarray(z, dtype=np.float32)
    W = np.asarray(W, dtype=np.float32)
    be = np.asarray(batch_edges)

    z_pad = np.zeros((N_PAD, W_DIM), dtype=np.float32)
    z_pad[:N_NODES] = z
    z16 = z_pad.astype(np.float16)
    w_np = W.astype(np.float16)

    rows_all = be[0].astype(np.int64)
    cols_all = be[1].astype(np.int64)
    glob_order = np.argsort(rows_all, kind="stable")

    # Shared window size across cores/groups (baked into the program).
    SW = 16
    per_core = []
    for c in range(N_CORES):
        sel = glob_order[c * EC:(c + 1) * EC]
        r_s = rows_all[sel]
        c_s = cols_all[sel]
        rg = r_s.reshape(NG, G_EDGES)
        w0 = rg[:, 0]
        span = int((rg[:, -1] - w0 + 1).max())
        SW = max(SW, span)
        per_core.append((r_s, c_s, w0))
    SW = (SW + 15) // 16 * 16
    assert SW <= 64, f"row-window {SW} exceeds 64 (PSUM bank limit)"

    in_maps = []
    for c in range(N_CORES):
        r_s, c_s, w0 = per_core[c]
        # zt2 segments: group g = z rows [w0_g, w0_g+SW), transposed
        zt2 = np.zeros((128, 4, NG * SW), np.float16)
        for g in range(NG):
            a = int(w0[g])
            seg = z16[a:a + SW]
            if seg.shape[0] < SW:  # clip at table end (can't happen: rows<10000)
                seg = np.pad(seg, ((0, SW - seg.shape[0]), (0, 0)))
            zt2[:, :, g * SW:(g + 1) * SW] = seg.reshape(SW, 4, 128).transpose(2, 1, 0)
        # one-hot masks: mask[p, b, w] = (rl[b*128+p] == w)
        rl = (r_s - np.repeat(w0, G_EDGES)).astype(np.int64)  # [EC] in [0, SW)
        assert rl.min() >= 0 and rl.max() < SW
        mk = np.zeros((NBLK * 128, SW), np.float16)
        mk[np.arange(EC), rl] = 1.0
        mk = np.ascontiguousarray(
            mk.reshape(NBLK, 128, SW).transpose(1, 0, 2))  # [128, NBLK, SW]
        cpad = np.zeros(EC_PAD, np.int16)
        cpad[:EC] = c_s.astype(np.int16)
        # host-pre-gathered transposed col tiles for the first KPRE chunks
        pre = z16[c_s[: KPRE * TNI]].reshape(KPRE, TNI, 4, 128).transpose(0, 3, 2, 1)
        in_maps.append({
            "zt2": np.ascontiguousarray(zt2),
            "ztbl": z16,
            "w": w_np,
            "cidx": _wrap_idx(cpad),
            "masks": mk,
            "ctpre": np.ascontiguousarray(pre),
        })

    nc = _get_nc(SW)
    kwargs = {"trace": True} if _profile else {}
    res = run_bass_kernel_spmd(nc, in_maps, core_ids=list(range(N_CORES)), **kwargs)
    _cache["last_res"] = res

    chunks = []
    for c in range(N_CORES):
        sc = res.results[c]["scores"]  # [128, NBLK], edge b*128+p at [p, b]
        chunks.append(np.ascontiguousarray(sc.T).reshape(-1))
    full = np.empty(N_EDGES, dtype=np.float32)
    full[glob_order] = np.concatenate(chunks)
    return full
